# revision 43
# baseline (speedup 1.0000x reference)
"""Dequantized mixed-sign int8 GEMM on 8 trn2 NeuronCores.

out = ((x - X_ZP) * X_SCALE) @ ((y - Y_ZP) * Y_SCALE)   [4096 x 4096 x 4096]

Strategy (fp8 path): the shifted operands (x+66 in [-62,193], y-160 in
[-160,95]) fit inside fp8-e4m3 range (+-240).  Rounding them to e4m3
costs ~2.3e-3 relative error on the output (vs the 2e-2 gate) because
the per-element relative rounding error (2^-4) averages down over the
K=4096 contraction while the output magnitude is dominated by the exact
mean term.  The GEMM then runs as fp8 x fp8 with DoubleRow perf mode
(two 128-row K-subtiles contracted per instruction, 157 TF/s = 2x bf16
peak) accumulating into fp32 PSUM; the scale product lands in the
PSUM->SBUF copy.  Measured ~127us vs the ~218us bf16 roofline
(~113us of pure matmul at fp8 peak + ~7us fixed NEFF preamble + ~3us
startup DMA latency + ~5us drain tail).

Sharding: 4-way over M x 2-way over N (core (mi, nj), mi in 0..3,
nj in 0..1).  Each core gets x[mi].T and y[:, nj] pre-shifted, cast to
fp8 and pre-tiled on the host into the DoubleRow SBUF layout
([128, 2, free] K-pair groups) so every DMA is one contiguous block and
there is zero on-chip elementwise work before the matmul.  x.T lives
SBUF-resident; y streams through in 512-wide blocks, double-buffered,
with 8 concurrent PSUM accumulation chains (one per PSUM bank).

Schedule notes (measured on hw, each worth ~1-2us):
- y loads ride the sync-engine HWDGE ring, x loads + output drains ride
  the ACT-engine ring, so descriptors issue in parallel and output-drain
  semaphore waits never stall the y stream.
- 5 dummy matmuls on a zeroed scratch tile warm the PE out of its low
  power-state during the otherwise-dead first-data window (the pstate
  decays when idle, so they must run back-to-back into the real stream).
- x group 0 splits in halves across both rings so the first matmul
  waits on 256KB, not 384KB.
- Block 0 interleaves all 8 chains per K step (the stream is input-
  paced); later blocks run chain-at-a-time so PSUM drains stagger.

Fallback: if the inputs are not integer-valued in the expected range
(so the fp8 error model would not hold), fall back to the exact bf16
kernel, and to a plain fp32 kernel if even bf16 would round.
"""

import sys

if "/opt/trn_rl_repo" not in sys.path:
    sys.path.insert(0, "/opt/trn_rl_repo")

import numpy as np

X_SCALE, X_ZP = 0.03, -66.0
Y_SCALE, Y_ZP = 0.025, 160.0
OUT_SCALE = float(np.float32(X_SCALE) * np.float32(Y_SCALE))

M = K = N = 4096
MI, NJ = 4, 2  # core grid: M split x N split
M_SH, N_SH = M // MI, N // NJ  # 1024, 2048 per core
N_CORES = MI * NJ
NBW = 512  # n-block width (one PSUM bank of fp32)


def build_fp8(m_sh=M_SH, n_sh=N_SH, k=K, nbw=NBW):
    """fp8-e4m3 DoubleRow GEMM.  Inputs arrive pre-shifted ((x - X_ZP),
    (y - Y_ZP)) and pre-tiled into K-pair groups [128, 2, free] so each
    matmul instruction contracts 256 K rows in 512 cycles (2x bf16
    FLOPs/cycle; measured 0.215us per [256K x 128M x 512N] matmul).
    No on-chip elementwise work: DVE only drains PSUM (scale by
    OUT_SCALE) and DMA moves raw blocks."""
    from concourse import bacc, mybir, tile

    f32, fp8 = mybir.dt.float32, mybir.dt.float8e4
    bf16 = mybir.dt.bfloat16
    DR = mybir.MatmulPerfMode.DoubleRow
    kq = k // 256  # K-pair groups (one DoubleRow matmul each)
    mo_n = m_sh // 128  # output chains (PSUM banks)
    nb_n = n_sh // nbw  # N blocks

    nc = bacc.Bacc("TRN2", target_bir_lowering=False, debug=False)
    xt_d = nc.dram_tensor("xt", (kq, 128, 2, m_sh), fp8, kind="ExternalInput")
    y_d = nc.dram_tensor("y", (nb_n, kq, 128, 2, nbw), fp8, kind="ExternalInput")
    # outputs ship bf16 (host upcasts): ~0.3us faster drain tail measured,
    # ~0.1% extra rounding against an 8.8x error-budget margin
    o_d = nc.dram_tensor("o", (mo_n, nb_n, 128, nbw), bf16, kind="ExternalOutput")

    with tile.TileContext(nc) as tc:
        with (
            tc.tile_pool(name="xp", bufs=1) as xp,
            tc.tile_pool(name="yp", bufs=3) as yp,
            tc.tile_pool(name="op", bufs=4) as op,
            tc.tile_pool(name="psum", bufs=1, space="PSUM") as pp,
        ):
            def load_y_tile(nb, g):
                yb = yp.tile([128, 2, nbw], fp8, tag=f"y{g}", name=f"y{nb}_{g}")
                nc.sync.dma_start(yb[:], y_d.ap()[nb, g])
                return yb

            # PE pstate warm-up: the tensor engine ramps from a low power
            # state and decays back when idle, so the dummies must run
            # back-to-back into the start of the real stream.  Fill the
            # dead window (first operands land ~1.8us after the first DMA
            # descriptor) with full-size dummy matmuls on a zeroed
            # scratch tile (into the ps7 bank, closed groups, numerically
            # inert) so the real stream starts already ramped.
            # small scratch: its memset gates the first dummy, and the
            # smaller dummy quantum wastes less time straddling the
            # moment real data lands
            scratch = xp.tile([128, 2, 128], fp8, tag="warm", name="warm")
            nc.gpsimd.memset(scratch[:], 0.0)
            ps_warm = pp.tile([128, nbw], f32, tag="ps7", name="ps7w")
            for _ in range(8):
                nc.tensor.matmul(
                    ps_warm[:, :128],
                    scratch[:],
                    scratch[:],
                    start=True,
                    stop=True,
                    perf_mode=DR,
                )

            # Startup stream: y descriptors issue on the sync ring while
            # x descriptors issue in parallel on the ACT ring (both are
            # HWDGE queues), so the first matmul's operands (y block-0
            # slice + resident x group 0) are in flight simultaneously
            # instead of serialized behind one descriptor queue.  The
            # g=0 x tile additionally splits across BOTH rings (half
            # right behind y block-0 g=0 on sync, half leading the ACT
            # ring) so its last byte lands ~1us sooner.  NOTE: tile
            # allocation order (y/x interleaved) is deliberate — grouping
            # all y tiles below all x tiles costs ~20% steady-state
            # matmul rate to SBUF bank conflicts.
            xbs = []
            y0 = []
            y0.append(load_y_tile(0, 0))
            x0a = xp.tile([128, 2, m_sh // 2], fp8, tag="x0a", name="x0a")
            nc.scalar.dma_start(x0a[:], xt_d.ap()[0][:, :, : m_sh // 2])
            x0b = xp.tile([128, 2, m_sh // 2], fp8, tag="x0b", name="x0b")
            nc.sync.dma_start(x0b[:], xt_d.ap()[0][:, :, m_sh // 2 :])
            xbs.append((x0a, x0b))
            for g in range(1, kq):
                y0.append(load_y_tile(0, g))
                xb = xp.tile([128, 2, m_sh], fp8, tag=f"x{g}", name=f"x{g}")
                nc.scalar.dma_start(xb[:], xt_d.ap()[g])
                xbs.append(xb)

            def x_slice(g, mo):
                if g == 0:
                    half = xbs[0][mo // 4]
                    return half[:, :, 128 * (mo % 4) : 128 * (mo % 4 + 1)]
                return xbs[g][:, :, 128 * mo : 128 * (mo + 1)]

            ybs_next = (
                [load_y_tile(1, g) for g in range(kq)] if nb_n > 1 else None
            )
            ybs = y0
            for nb in range(nb_n):
                pss = [
                    pp.tile([128, nbw], f32, tag=f"ps{mo}", name=f"ps{mo}")
                    for mo in range(mo_n)
                ]

                def copy_out(mo, nb=nb):
                    # output drains ride the ACT descriptor ring: their
                    # semaphore waits never stall the sync ring feeding
                    # the y stream, and the final drain's descriptor is
                    # pre-positioned on an otherwise idle queue
                    ot = op.tile([128, nbw], bf16, tag="ot", name="ot")
                    nc.vector.tensor_scalar_mul(ot[:], pss[mo][:], OUT_SCALE)
                    nc.scalar.dma_start(o_d.ap()[mo, nb], ot[:])

                def mm(mo, g, start=None, ybs=ybs):
                    nc.tensor.matmul(
                        pss[mo][:],
                        x_slice(g, mo),
                        ybs[g][:],
                        start=(g == 0) if start is None else start,
                        stop=(g == kq - 1),
                        perf_mode=DR,
                    )

                if nb == 0:
                    # block 0 is paced by the input stream: interleave all
                    # chains per K step so every arriving K-pair group
                    # unlocks mo_n matmuls for the in-order PE
                    for g in range(kq):
                        for mo in range(mo_n):
                            mm(mo, g)
                    for mo in range(mo_n):
                        copy_out(mo)
                else:
                    # data resident: run chains to completion one at a
                    # time so completions (and PSUM copies) stagger
                    # through the block instead of bursting at its end
                    for mo in range(mo_n):
                        for g in range(kq):
                            mm(mo, g)
                        copy_out(mo)
                ybs = ybs_next
                ybs_next = (
                    [load_y_tile(nb + 2, g) for g in range(kq)]
                    if nb + 2 < nb_n
                    else None
                )

    nc.compile()
    return nc


def build(m_sh=M_SH, n_sh=N_SH, k=K, nbw=NBW):
    """Fallback fp32 variant (used only if inputs are not even bf16-exact)."""
    from concourse import bacc, mybir, tile

    f32, bf16 = mybir.dt.float32, mybir.dt.bfloat16
    kp = k // 128  # K tiles of 128
    mo_n = m_sh // 128  # M tiles of 128
    nb_n = n_sh // nbw  # N blocks

    nc = bacc.Bacc("TRN2", target_bir_lowering=False, debug=False)
    xt_d = nc.dram_tensor("xt", (k, m_sh), f32, kind="ExternalInput")
    y_d = nc.dram_tensor("y", (k, n_sh), f32, kind="ExternalInput")
    o_d = nc.dram_tensor("o", (m_sh, n_sh), f32, kind="ExternalOutput")

    with tile.TileContext(nc) as tc:
        with (
            tc.tile_pool(name="xstage", bufs=3) as xstage,
            tc.tile_pool(name="ystage", bufs=8) as ystage,
            tc.tile_pool(name="xbf", bufs=1) as xbfp,
            tc.tile_pool(name="ybf", bufs=2) as ybfp,
            tc.tile_pool(name="opool", bufs=4) as opool,
            tc.tile_pool(name="psum", bufs=1, space="PSUM") as psum,
        ):
            def load_y(nb, ko):
                ys = ystage.tile([128, nbw], f32, tag="ys")
                nc.sync.dma_start(
                    ys[:],
                    y_d.ap()[128 * ko : 128 * (ko + 1), nb * nbw : (nb + 1) * nbw],
                )
                yb = ybfp.tile([128, nbw], bf16, tag=f"y{ko}")
                nc.vector.tensor_scalar_add(yb[:], ys[:], -Y_ZP)
                return yb

            xbf = []
            ybs0 = []
            for ko in range(kp):
                ybs0.append(load_y(0, ko))
                xs = xstage.tile([128, m_sh], f32, tag="xs")
                nc.sync.dma_start(xs[:], xt_d.ap()[128 * ko : 128 * (ko + 1), :])
                xb = xbfp.tile([128, m_sh], bf16, tag=f"x{ko}")
                nc.scalar.activation(
                    xb[:], xs[:], mybir.ActivationFunctionType.Copy, bias=-X_ZP
                )
                xbf.append(xb)

            for nb in range(nb_n):
                ybs = ybs0 if nb == 0 else [load_y(nb, ko) for ko in range(kp)]

                pss = [
                    psum.tile([128, nbw], f32, tag=f"ps{mo}", name=f"ps{mo}")
                    for mo in range(mo_n)
                ]
                for ko in range(kp):
                    for mo in range(mo_n):
                        nc.tensor.matmul(
                            pss[mo][:],
                            xbf[ko][:, 128 * mo : 128 * (mo + 1)],
                            ybs[ko][:],
                            start=(ko == 0),
                            stop=(ko == kp - 1),
                        )
                for mo in range(mo_n):
                    ot = opool.tile([128, nbw], f32, tag="ot")
                    nc.scalar.activation(
                        ot[:], pss[mo][:], mybir.ActivationFunctionType.Copy,
                        scale=OUT_SCALE,
                    )
                    nc.sync.dma_start(
                        o_d.ap()[128 * mo : 128 * (mo + 1), nb * nbw : (nb + 1) * nbw],
                        ot[:],
                    )

    nc.compile()
    return nc


def build_bf16(m_sh=M_SH, n_sh=N_SH, k=K, nbw=NBW):
    """bf16-input variant: exact for integer-valued quantized data; used
    as fallback when fp8's error model does not apply but bf16 casts are
    lossless."""
    from concourse import bacc, mybir, tile

    f32, bf16 = mybir.dt.float32, mybir.dt.bfloat16
    kp = k // 128
    mo_n = m_sh // 128
    nb_n = n_sh // nbw
    xg_n = kp // 2  # x groups: [128, 2, m_sh] (two K tiles per load)
    yg_n = kp // 4  # y groups: [128, 4, nbw] (four K tiles per load)

    nc = bacc.Bacc("TRN2", target_bir_lowering=False, debug=False)
    xt_d = nc.dram_tensor("xt", (xg_n, 128, 2, m_sh), bf16, kind="ExternalInput")
    y_d = nc.dram_tensor("y", (nb_n, yg_n, 128, 4, nbw), bf16, kind="ExternalInput")
    o_d = nc.dram_tensor("o", (mo_n, nb_n, 128, nbw), f32, kind="ExternalOutput")

    with tile.TileContext(nc) as tc:
        with (
            tc.tile_pool(name="xbf", bufs=1) as xbfp,
            tc.tile_pool(name="ybf", bufs=3) as ybfp,
            tc.tile_pool(name="opool", bufs=4) as opool,
            tc.tile_pool(name="psum", bufs=1, space="PSUM") as psum,
        ):
            x_at = [None] * kp
            y0_at = [None] * kp

            def load_x(g, eng_dve):
                xb = xbfp.tile([128, 2, m_sh], bf16, tag=f"x{g}", name=f"x{g}")
                nc.sync.dma_start(xb[:], xt_d.ap()[g])
                if eng_dve:
                    nc.vector.tensor_scalar_add(xb[:], xb[:], -X_ZP)
                else:
                    nc.scalar.activation(
                        xb[:], xb[:], mybir.ActivationFunctionType.Copy, bias=-X_ZP
                    )
                x_at[2 * g] = (xb, 0)
                x_at[2 * g + 1] = (xb, 1)

            def load_y(nb):
                tiles = []
                for g in range(yg_n):
                    yb = ybfp.tile(
                        [128, 4, nbw], bf16, tag=f"y{g}", name=f"y{nb}_{g}"
                    )
                    nc.sync.dma_start(yb[:], y_d.ap()[nb, g])
                    nc.vector.tensor_scalar_add(yb[:], yb[:], -Y_ZP)
                    tiles.append(yb)
                return tiles

            def x_slice(ko, mo):  # lhsT [128, 128]
                xb, j = x_at[ko]
                return xb[:, j, 128 * mo : 128 * (mo + 1)]

            def y_slice(ybs, ko):  # rhs [128, nbw]
                if ybs is None:
                    yb, j = y0_at[ko]
                    return yb[:, j, :]
                g, j = divmod(ko, 4)
                return ybs[g][:, j, :]

            def load_y0_part(j0, j1, tag):
                yb = ybfp.tile([128, j1 - j0, nbw], bf16, tag=tag, name=tag)
                nc.sync.dma_start(yb[:], y_d.ap()[0, 0][:, j0:j1, :])
                nc.vector.tensor_scalar_add(yb[:], yb[:], -Y_ZP)
                for j in range(j0, j1):
                    y0_at[j] = (yb, j - j0)

            def load_x0_part(j, tag, eng_dve):
                xb = xbfp.tile([128, 1, m_sh], bf16, tag=tag, name=tag)
                nc.sync.dma_start(xb[:], xt_d.ap()[0][:, j : j + 1, :])
                if eng_dve:
                    nc.vector.tensor_scalar_add(xb[:], xb[:], -X_ZP)
                else:
                    nc.scalar.activation(
                        xb[:], xb[:], mybir.ActivationFunctionType.Copy, bias=-X_ZP
                    )
                x_at[j] = (xb, 0)

            load_y0_part(0, 1, "y0a")
            load_x0_part(0, "x0a", eng_dve=True)
            load_x0_part(1, "x0b", eng_dve=False)
            if kp > 1:
                load_y0_part(1, min(4, kp), "y0b")
            if xg_n > 1:
                load_x(1, eng_dve=True)
            for gg in range(1, yg_n):
                yb = ybfp.tile([128, 4, nbw], bf16, tag=f"y{gg}", name=f"y0_{gg}")
                nc.sync.dma_start(yb[:], y_d.ap()[0, gg])
                nc.vector.tensor_scalar_add(yb[:], yb[:], -Y_ZP)
                for j in range(4):
                    y0_at[4 * gg + j] = (yb, j)
                for g in (2 * gg, 2 * gg + 1):
                    if g < xg_n:
                        load_x(g, eng_dve=(g % 2 == 0))

            ybs_next = load_y(1) if nb_n > 1 else None
            ybs = None
            for nb in range(nb_n):
                pss = [
                    psum.tile([128, nbw], f32, tag=f"ps{mo}", name=f"ps{mo}")
                    for mo in range(mo_n)
                ]

                def copy_out(mo, nb=nb):
                    ot = opool.tile([128, nbw], f32, tag="ot", name="ot")
                    use_dve = mo % 2 == 0 or (
                        nb == nb_n - 1 and mo == mo_n - 1
                    )
                    if use_dve:
                        nc.vector.tensor_scalar_mul(ot[:], pss[mo][:], OUT_SCALE)
                    else:
                        nc.scalar.activation(
                            ot[:], pss[mo][:], mybir.ActivationFunctionType.Copy,
                            scale=OUT_SCALE,
                        )
                    nc.sync.dma_start(o_d.ap()[mo, nb], ot[:])

                if nb == 0:
                    for ko in range(kp):
                        for mo in range(mo_n):
                            nc.tensor.matmul(
                                pss[mo][:],
                                x_slice(ko, mo),
                                y_slice(ybs, ko),
                                start=(ko == 0),
                                stop=(ko == kp - 1),
                            )
                    for mo in range(mo_n):
                        copy_out(mo)
                else:
                    for mo in range(mo_n):
                        for ko in range(kp):
                            nc.tensor.matmul(
                                pss[mo][:],
                                x_slice(ko, mo),
                                y_slice(ybs, ko),
                                start=(ko == 0),
                                stop=(ko == kp - 1),
                            )
                        copy_out(mo)
                ybs = ybs_next
                ybs_next = load_y(nb + 2) if nb + 2 < nb_n else None

    nc.compile()
    return nc


_nc_cache = {}


def _get_nc(variant="fp8"):
    if variant not in _nc_cache:
        builders = {"fp8": build_fp8, "bf16": build_bf16, "f32": build}
        _nc_cache[variant] = builders[variant]()
    return _nc_cache[variant]


def _fp8_applicable(x: np.ndarray, y: np.ndarray) -> bool:
    """fp8 path needs integer-valued inputs whose shifted values fit well
    inside e4m3 range (so the rounding-error model holds)."""
    for a, zp in ((x, X_ZP), (y, Y_ZP)):
        if not np.array_equal(a, np.rint(a)):
            return False
        s = a - zp
        if np.abs(s).max() > 240.0:
            return False
    return True


def make_in_maps_fp8(x: np.ndarray, y: np.ndarray) -> list[dict]:
    """Shift, cast to fp8-e4m3 and pre-tile into the DoubleRow layouts.

    xt: per core [K, M_SH] -> [K/256, 128, 2, M_SH]
        (k = 256*g + 128*i + p  ->  [g, p, i, :])
    y:  per core [K, N_SH] -> [NB, K/256, 128, 2, NBW]  (same k bijection)
    """
    import ml_dtypes

    f8 = ml_dtypes.float8_e4m3
    xs = (np.ascontiguousarray(x, dtype=np.float32) - X_ZP).astype(f8)
    ys = (np.ascontiguousarray(y, dtype=np.float32) - Y_ZP).astype(f8)
    kq = K // 256
    nb_n = N_SH // NBW
    xt_shards = []
    for mi in range(MI):
        xt = xs[mi * M_SH : (mi + 1) * M_SH].T  # [K, M_SH]
        t = xt.reshape(kq, 2, 128, M_SH).transpose(0, 2, 1, 3)
        xt_shards.append(np.ascontiguousarray(t))
    y_shards = []
    for nj in range(NJ):
        yc = ys[:, nj * N_SH : (nj + 1) * N_SH]  # [K, N_SH]
        t = yc.reshape(kq, 2, 128, nb_n, NBW).transpose(3, 0, 2, 1, 4)
        y_shards.append(np.ascontiguousarray(t))
    return [{"xt": xt_shards[i // NJ], "y": y_shards[i % NJ]} for i in range(N_CORES)]


def make_in_maps(x: np.ndarray, y: np.ndarray) -> list[dict]:
    x = np.ascontiguousarray(x, dtype=np.float32)
    y = np.ascontiguousarray(y, dtype=np.float32)
    xt_shards = [
        np.ascontiguousarray(x[mi * M_SH : (mi + 1) * M_SH].T) for mi in range(MI)
    ]
    y_shards = [
        np.ascontiguousarray(y[:, nj * N_SH : (nj + 1) * N_SH]) for nj in range(NJ)
    ]
    return [{"xt": xt_shards[i // NJ], "y": y_shards[i % NJ]} for i in range(N_CORES)]


def make_in_maps_bf16(xb: np.ndarray, yb: np.ndarray) -> list[dict]:
    """Pre-tile bf16 shards to match build_bf16's DRAM layouts."""
    kp = K // 128
    nb_n = N_SH // NBW
    xt_shards = []
    for mi in range(MI):
        xt = xb[mi * M_SH : (mi + 1) * M_SH].T  # [K, M_SH]
        t = xt.reshape(kp // 2, 2, 128, M_SH).transpose(0, 2, 1, 3)
        xt_shards.append(np.ascontiguousarray(t))
    y_shards = []
    for nj in range(NJ):
        ys = yb[:, nj * N_SH : (nj + 1) * N_SH]  # [K, N_SH]
        t = ys.reshape(kp // 4, 4, 128, nb_n, NBW).transpose(3, 0, 2, 1, 4)
        y_shards.append(np.ascontiguousarray(t))
    return [{"xt": xt_shards[i // NJ], "y": y_shards[i % NJ]} for i in range(N_CORES)]


def _cast_bf16_exact(x: np.ndarray, y: np.ndarray):
    """Lossless repack to bf16 when every value survives the cast."""
    import ml_dtypes

    xb = np.ascontiguousarray(x, dtype=np.float32).astype(ml_dtypes.bfloat16)
    yb = np.ascontiguousarray(y, dtype=np.float32).astype(ml_dtypes.bfloat16)
    if np.array_equal(xb.astype(np.float32), x) and np.array_equal(
        yb.astype(np.float32), y
    ):
        return xb, yb
    return None


def _plan(x: np.ndarray, y: np.ndarray):
    """Pick the kernel variant + host-packed input maps for these inputs."""
    if _fp8_applicable(x, y):
        return "fp8", make_in_maps_fp8(x, y)
    casted = _cast_bf16_exact(x, y)
    if casted is not None:
        return "bf16", make_in_maps_bf16(*casted)
    return "f32", make_in_maps(x, y)


def kernel(x: np.ndarray, y: np.ndarray) -> np.ndarray:
    from concourse import bass_utils

    variant, in_maps = _plan(x, y)
    nc = _get_nc(variant)

    res = bass_utils.run_bass_kernel_spmd(nc, in_maps, core_ids=list(range(N_CORES)))

    out = np.empty((M, N), dtype=np.float32)
    for i in range(N_CORES):
        mi, nj = i // NJ, i % NJ
        o = res.results[i]["o"]
        if o.ndim == 4:  # [MO, NB, 128, NBW] pre-tiled layout
            o = o.transpose(0, 2, 1, 3).reshape(M_SH, N_SH)
        out[mi * M_SH : (mi + 1) * M_SH, nj * N_SH : (nj + 1) * N_SH] = o
    return out


# revision 44
# speedup vs baseline: 1.0034x; 1.0034x over previous
"""Dequantized mixed-sign int8 GEMM on 8 trn2 NeuronCores.

out = ((x - X_ZP) * X_SCALE) @ ((y - Y_ZP) * Y_SCALE)   [4096 x 4096 x 4096]

Strategy (fp8 path): the shifted operands (x+66 in [-62,193], y-160 in
[-160,95]) fit inside fp8-e4m3 range (+-240).  Rounding them to e4m3
costs ~2.3e-3 relative error on the output (vs the 2e-2 gate) because
the per-element relative rounding error (2^-4) averages down over the
K=4096 contraction while the output magnitude is dominated by the exact
mean term.  The GEMM then runs as fp8 x fp8 with DoubleRow perf mode
(two 128-row K-subtiles contracted per instruction, 157 TF/s = 2x bf16
peak) accumulating into fp32 PSUM; the scale product lands in the
PSUM->SBUF copy.  Measured ~127us vs the ~218us bf16 roofline
(~113us of pure matmul at fp8 peak + ~7us fixed NEFF preamble + ~3us
startup DMA latency + ~5us drain tail).

Sharding: 4-way over M x 2-way over N (core (mi, nj), mi in 0..3,
nj in 0..1).  Each core gets x[mi].T and y[:, nj] pre-shifted, cast to
fp8 and pre-tiled on the host into the DoubleRow SBUF layout
([128, 2, free] K-pair groups) so every DMA is one contiguous block and
there is zero on-chip elementwise work before the matmul.  x.T lives
SBUF-resident; y streams through in 512-wide blocks, double-buffered,
with 8 concurrent PSUM accumulation chains (one per PSUM bank).

Schedule notes (measured on hw, each worth ~1-2us):
- y loads ride the sync-engine HWDGE ring, x loads + output drains ride
  the ACT-engine ring, so descriptors issue in parallel and output-drain
  semaphore waits never stall the y stream.
- 5 dummy matmuls on a zeroed scratch tile warm the PE out of its low
  power-state during the otherwise-dead first-data window (the pstate
  decays when idle, so they must run back-to-back into the real stream).
- x group 0 splits in halves across both rings so the first matmul
  waits on 256KB, not 384KB.
- Block 0 interleaves all 8 chains per K step (the stream is input-
  paced); later blocks run chain-at-a-time so PSUM drains stagger.

Fallback: if the inputs are not integer-valued in the expected range
(so the fp8 error model would not hold), fall back to the exact bf16
kernel, and to a plain fp32 kernel if even bf16 would round.
"""

import sys

if "/opt/trn_rl_repo" not in sys.path:
    sys.path.insert(0, "/opt/trn_rl_repo")

import numpy as np

X_SCALE, X_ZP = 0.03, -66.0
Y_SCALE, Y_ZP = 0.025, 160.0
OUT_SCALE = float(np.float32(X_SCALE) * np.float32(Y_SCALE))

M = K = N = 4096
MI, NJ = 4, 2  # core grid: M split x N split
M_SH, N_SH = M // MI, N // NJ  # 1024, 2048 per core
N_CORES = MI * NJ
NBW = 512  # n-block width (one PSUM bank of fp32)


def build_fp8(m_sh=M_SH, n_sh=N_SH, k=K, nbw=NBW):
    """fp8-e4m3 DoubleRow GEMM.  Inputs arrive pre-shifted ((x - X_ZP),
    (y - Y_ZP)) and pre-tiled into K-pair groups [128, 2, free] so each
    matmul instruction contracts 256 K rows in 512 cycles (2x bf16
    FLOPs/cycle; measured 0.215us per [256K x 128M x 512N] matmul).
    No on-chip elementwise work: DVE only drains PSUM (scale by
    OUT_SCALE) and DMA moves raw blocks."""
    from concourse import bacc, mybir, tile

    f32, fp8 = mybir.dt.float32, mybir.dt.float8e4
    bf16 = mybir.dt.bfloat16
    DR = mybir.MatmulPerfMode.DoubleRow
    kq = k // 256  # K-pair groups (one DoubleRow matmul each)
    mo_n = m_sh // 128  # output chains (PSUM banks)
    nb_n = n_sh // nbw  # N blocks

    nc = bacc.Bacc("TRN2", target_bir_lowering=False, debug=False)
    xt_d = nc.dram_tensor("xt", (kq, 128, 2, m_sh), fp8, kind="ExternalInput")
    y_d = nc.dram_tensor("y", (nb_n, kq, 128, 2, nbw), fp8, kind="ExternalInput")
    # outputs ship bf16 (host upcasts): ~0.3us faster drain tail measured,
    # ~0.1% extra rounding against an 8.8x error-budget margin
    o_d = nc.dram_tensor("o", (mo_n, nb_n, 128, nbw), bf16, kind="ExternalOutput")

    with tile.TileContext(nc) as tc:
        with (
            tc.tile_pool(name="xp", bufs=1) as xp,
            tc.tile_pool(name="yp", bufs=3) as yp,
            tc.tile_pool(name="op", bufs=4) as op,
            tc.tile_pool(name="psum", bufs=1, space="PSUM") as pp,
        ):
            def load_y_tile(nb, g):
                yb = yp.tile([128, 2, nbw], fp8, tag=f"y{g}", name=f"y{nb}_{g}")
                nc.sync.dma_start(yb[:], y_d.ap()[nb, g])
                return yb

            # PE pstate warm-up: the tensor engine ramps from a low power
            # state and decays back when idle, so the dummies must run
            # back-to-back into the start of the real stream.  Fill the
            # dead window (first operands land ~1.8us after the first DMA
            # descriptor) with full-size dummy matmuls on a zeroed
            # scratch tile (into the ps7 bank, closed groups, numerically
            # inert) so the real stream starts already ramped.
            # small scratch: its memset gates the first dummy, and the
            # smaller dummy quantum wastes less time straddling the
            # moment real data lands
            scratch = xp.tile([128, 2, 128], fp8, tag="warm", name="warm")
            nc.gpsimd.memset(scratch[:], 0.0)
            ps_warm = pp.tile([128, nbw], f32, tag="ps7", name="ps7w")
            for _ in range(16):
                nc.tensor.matmul(
                    ps_warm[:, :128],
                    scratch[:],
                    scratch[:],
                    start=True,
                    stop=True,
                    perf_mode=DR,
                )

            # Startup stream: y descriptors issue on the sync ring while
            # x descriptors issue in parallel on the ACT ring (both are
            # HWDGE queues), so the first matmul's operands (y block-0
            # slice + resident x group 0) are in flight simultaneously
            # instead of serialized behind one descriptor queue.  The
            # g=0 x tile additionally splits across BOTH rings (half
            # right behind y block-0 g=0 on sync, half leading the ACT
            # ring) so its last byte lands ~1us sooner.  NOTE: tile
            # allocation order (y/x interleaved) is deliberate — grouping
            # all y tiles below all x tiles costs ~20% steady-state
            # matmul rate to SBUF bank conflicts.
            xbs = []
            y0 = []
            y0.append(load_y_tile(0, 0))
            x0a = xp.tile([128, 2, m_sh // 2], fp8, tag="x0a", name="x0a")
            nc.scalar.dma_start(x0a[:], xt_d.ap()[0][:, :, : m_sh // 2])
            x0b = xp.tile([128, 2, m_sh // 2], fp8, tag="x0b", name="x0b")
            nc.sync.dma_start(x0b[:], xt_d.ap()[0][:, :, m_sh // 2 :])
            xbs.append((x0a, x0b))
            for g in range(1, kq):
                y0.append(load_y_tile(0, g))
                xb = xp.tile([128, 2, m_sh], fp8, tag=f"x{g}", name=f"x{g}")
                nc.scalar.dma_start(xb[:], xt_d.ap()[g])
                xbs.append(xb)

            def x_slice(g, mo):
                if g == 0:
                    half = xbs[0][mo // 4]
                    return half[:, :, 128 * (mo % 4) : 128 * (mo % 4 + 1)]
                return xbs[g][:, :, 128 * mo : 128 * (mo + 1)]

            ybs_next = (
                [load_y_tile(1, g) for g in range(kq)] if nb_n > 1 else None
            )
            ybs = y0
            for nb in range(nb_n):
                pss = [
                    pp.tile([128, nbw], f32, tag=f"ps{mo}", name=f"ps{mo}")
                    for mo in range(mo_n)
                ]

                def copy_out(mo, nb=nb):
                    # output drains ride the ACT descriptor ring: their
                    # semaphore waits never stall the sync ring feeding
                    # the y stream, and the final drain's descriptor is
                    # pre-positioned on an otherwise idle queue
                    ot = op.tile([128, nbw], bf16, tag="ot", name="ot")
                    nc.vector.tensor_scalar_mul(ot[:], pss[mo][:], OUT_SCALE)
                    nc.scalar.dma_start(o_d.ap()[mo, nb], ot[:])

                def mm(mo, g, start=None, ybs=ybs):
                    nc.tensor.matmul(
                        pss[mo][:],
                        x_slice(g, mo),
                        ybs[g][:],
                        start=(g == 0) if start is None else start,
                        stop=(g == kq - 1),
                        perf_mode=DR,
                    )

                if nb == 0:
                    # block 0 is paced by the input stream: interleave all
                    # chains per K step so every arriving K-pair group
                    # unlocks mo_n matmuls for the in-order PE
                    for g in range(kq):
                        for mo in range(mo_n):
                            mm(mo, g)
                    for mo in range(mo_n):
                        copy_out(mo)
                else:
                    # data resident: run chains to completion one at a
                    # time so completions (and PSUM copies) stagger
                    # through the block instead of bursting at its end
                    for mo in range(mo_n):
                        for g in range(kq):
                            mm(mo, g)
                        copy_out(mo)
                ybs = ybs_next
                ybs_next = (
                    [load_y_tile(nb + 2, g) for g in range(kq)]
                    if nb + 2 < nb_n
                    else None
                )

    nc.compile()
    return nc


def build(m_sh=M_SH, n_sh=N_SH, k=K, nbw=NBW):
    """Fallback fp32 variant (used only if inputs are not even bf16-exact)."""
    from concourse import bacc, mybir, tile

    f32, bf16 = mybir.dt.float32, mybir.dt.bfloat16
    kp = k // 128  # K tiles of 128
    mo_n = m_sh // 128  # M tiles of 128
    nb_n = n_sh // nbw  # N blocks

    nc = bacc.Bacc("TRN2", target_bir_lowering=False, debug=False)
    xt_d = nc.dram_tensor("xt", (k, m_sh), f32, kind="ExternalInput")
    y_d = nc.dram_tensor("y", (k, n_sh), f32, kind="ExternalInput")
    o_d = nc.dram_tensor("o", (m_sh, n_sh), f32, kind="ExternalOutput")

    with tile.TileContext(nc) as tc:
        with (
            tc.tile_pool(name="xstage", bufs=3) as xstage,
            tc.tile_pool(name="ystage", bufs=8) as ystage,
            tc.tile_pool(name="xbf", bufs=1) as xbfp,
            tc.tile_pool(name="ybf", bufs=2) as ybfp,
            tc.tile_pool(name="opool", bufs=4) as opool,
            tc.tile_pool(name="psum", bufs=1, space="PSUM") as psum,
        ):
            def load_y(nb, ko):
                ys = ystage.tile([128, nbw], f32, tag="ys")
                nc.sync.dma_start(
                    ys[:],
                    y_d.ap()[128 * ko : 128 * (ko + 1), nb * nbw : (nb + 1) * nbw],
                )
                yb = ybfp.tile([128, nbw], bf16, tag=f"y{ko}")
                nc.vector.tensor_scalar_add(yb[:], ys[:], -Y_ZP)
                return yb

            xbf = []
            ybs0 = []
            for ko in range(kp):
                ybs0.append(load_y(0, ko))
                xs = xstage.tile([128, m_sh], f32, tag="xs")
                nc.sync.dma_start(xs[:], xt_d.ap()[128 * ko : 128 * (ko + 1), :])
                xb = xbfp.tile([128, m_sh], bf16, tag=f"x{ko}")
                nc.scalar.activation(
                    xb[:], xs[:], mybir.ActivationFunctionType.Copy, bias=-X_ZP
                )
                xbf.append(xb)

            for nb in range(nb_n):
                ybs = ybs0 if nb == 0 else [load_y(nb, ko) for ko in range(kp)]

                pss = [
                    psum.tile([128, nbw], f32, tag=f"ps{mo}", name=f"ps{mo}")
                    for mo in range(mo_n)
                ]
                for ko in range(kp):
                    for mo in range(mo_n):
                        nc.tensor.matmul(
                            pss[mo][:],
                            xbf[ko][:, 128 * mo : 128 * (mo + 1)],
                            ybs[ko][:],
                            start=(ko == 0),
                            stop=(ko == kp - 1),
                        )
                for mo in range(mo_n):
                    ot = opool.tile([128, nbw], f32, tag="ot")
                    nc.scalar.activation(
                        ot[:], pss[mo][:], mybir.ActivationFunctionType.Copy,
                        scale=OUT_SCALE,
                    )
                    nc.sync.dma_start(
                        o_d.ap()[128 * mo : 128 * (mo + 1), nb * nbw : (nb + 1) * nbw],
                        ot[:],
                    )

    nc.compile()
    return nc


def build_bf16(m_sh=M_SH, n_sh=N_SH, k=K, nbw=NBW):
    """bf16-input variant: exact for integer-valued quantized data; used
    as fallback when fp8's error model does not apply but bf16 casts are
    lossless."""
    from concourse import bacc, mybir, tile

    f32, bf16 = mybir.dt.float32, mybir.dt.bfloat16
    kp = k // 128
    mo_n = m_sh // 128
    nb_n = n_sh // nbw
    xg_n = kp // 2  # x groups: [128, 2, m_sh] (two K tiles per load)
    yg_n = kp // 4  # y groups: [128, 4, nbw] (four K tiles per load)

    nc = bacc.Bacc("TRN2", target_bir_lowering=False, debug=False)
    xt_d = nc.dram_tensor("xt", (xg_n, 128, 2, m_sh), bf16, kind="ExternalInput")
    y_d = nc.dram_tensor("y", (nb_n, yg_n, 128, 4, nbw), bf16, kind="ExternalInput")
    o_d = nc.dram_tensor("o", (mo_n, nb_n, 128, nbw), f32, kind="ExternalOutput")

    with tile.TileContext(nc) as tc:
        with (
            tc.tile_pool(name="xbf", bufs=1) as xbfp,
            tc.tile_pool(name="ybf", bufs=3) as ybfp,
            tc.tile_pool(name="opool", bufs=4) as opool,
            tc.tile_pool(name="psum", bufs=1, space="PSUM") as psum,
        ):
            x_at = [None] * kp
            y0_at = [None] * kp

            def load_x(g, eng_dve):
                xb = xbfp.tile([128, 2, m_sh], bf16, tag=f"x{g}", name=f"x{g}")
                nc.sync.dma_start(xb[:], xt_d.ap()[g])
                if eng_dve:
                    nc.vector.tensor_scalar_add(xb[:], xb[:], -X_ZP)
                else:
                    nc.scalar.activation(
                        xb[:], xb[:], mybir.ActivationFunctionType.Copy, bias=-X_ZP
                    )
                x_at[2 * g] = (xb, 0)
                x_at[2 * g + 1] = (xb, 1)

            def load_y(nb):
                tiles = []
                for g in range(yg_n):
                    yb = ybfp.tile(
                        [128, 4, nbw], bf16, tag=f"y{g}", name=f"y{nb}_{g}"
                    )
                    nc.sync.dma_start(yb[:], y_d.ap()[nb, g])
                    nc.vector.tensor_scalar_add(yb[:], yb[:], -Y_ZP)
                    tiles.append(yb)
                return tiles

            def x_slice(ko, mo):  # lhsT [128, 128]
                xb, j = x_at[ko]
                return xb[:, j, 128 * mo : 128 * (mo + 1)]

            def y_slice(ybs, ko):  # rhs [128, nbw]
                if ybs is None:
                    yb, j = y0_at[ko]
                    return yb[:, j, :]
                g, j = divmod(ko, 4)
                return ybs[g][:, j, :]

            def load_y0_part(j0, j1, tag):
                yb = ybfp.tile([128, j1 - j0, nbw], bf16, tag=tag, name=tag)
                nc.sync.dma_start(yb[:], y_d.ap()[0, 0][:, j0:j1, :])
                nc.vector.tensor_scalar_add(yb[:], yb[:], -Y_ZP)
                for j in range(j0, j1):
                    y0_at[j] = (yb, j - j0)

            def load_x0_part(j, tag, eng_dve):
                xb = xbfp.tile([128, 1, m_sh], bf16, tag=tag, name=tag)
                nc.sync.dma_start(xb[:], xt_d.ap()[0][:, j : j + 1, :])
                if eng_dve:
                    nc.vector.tensor_scalar_add(xb[:], xb[:], -X_ZP)
                else:
                    nc.scalar.activation(
                        xb[:], xb[:], mybir.ActivationFunctionType.Copy, bias=-X_ZP
                    )
                x_at[j] = (xb, 0)

            load_y0_part(0, 1, "y0a")
            load_x0_part(0, "x0a", eng_dve=True)
            load_x0_part(1, "x0b", eng_dve=False)
            if kp > 1:
                load_y0_part(1, min(4, kp), "y0b")
            if xg_n > 1:
                load_x(1, eng_dve=True)
            for gg in range(1, yg_n):
                yb = ybfp.tile([128, 4, nbw], bf16, tag=f"y{gg}", name=f"y0_{gg}")
                nc.sync.dma_start(yb[:], y_d.ap()[0, gg])
                nc.vector.tensor_scalar_add(yb[:], yb[:], -Y_ZP)
                for j in range(4):
                    y0_at[4 * gg + j] = (yb, j)
                for g in (2 * gg, 2 * gg + 1):
                    if g < xg_n:
                        load_x(g, eng_dve=(g % 2 == 0))

            ybs_next = load_y(1) if nb_n > 1 else None
            ybs = None
            for nb in range(nb_n):
                pss = [
                    psum.tile([128, nbw], f32, tag=f"ps{mo}", name=f"ps{mo}")
                    for mo in range(mo_n)
                ]

                def copy_out(mo, nb=nb):
                    ot = opool.tile([128, nbw], f32, tag="ot", name="ot")
                    use_dve = mo % 2 == 0 or (
                        nb == nb_n - 1 and mo == mo_n - 1
                    )
                    if use_dve:
                        nc.vector.tensor_scalar_mul(ot[:], pss[mo][:], OUT_SCALE)
                    else:
                        nc.scalar.activation(
                            ot[:], pss[mo][:], mybir.ActivationFunctionType.Copy,
                            scale=OUT_SCALE,
                        )
                    nc.sync.dma_start(o_d.ap()[mo, nb], ot[:])

                if nb == 0:
                    for ko in range(kp):
                        for mo in range(mo_n):
                            nc.tensor.matmul(
                                pss[mo][:],
                                x_slice(ko, mo),
                                y_slice(ybs, ko),
                                start=(ko == 0),
                                stop=(ko == kp - 1),
                            )
                    for mo in range(mo_n):
                        copy_out(mo)
                else:
                    for mo in range(mo_n):
                        for ko in range(kp):
                            nc.tensor.matmul(
                                pss[mo][:],
                                x_slice(ko, mo),
                                y_slice(ybs, ko),
                                start=(ko == 0),
                                stop=(ko == kp - 1),
                            )
                        copy_out(mo)
                ybs = ybs_next
                ybs_next = load_y(nb + 2) if nb + 2 < nb_n else None

    nc.compile()
    return nc


_nc_cache = {}


def _get_nc(variant="fp8"):
    if variant not in _nc_cache:
        builders = {"fp8": build_fp8, "bf16": build_bf16, "f32": build}
        _nc_cache[variant] = builders[variant]()
    return _nc_cache[variant]


def _fp8_applicable(x: np.ndarray, y: np.ndarray) -> bool:
    """fp8 path needs integer-valued inputs whose shifted values fit well
    inside e4m3 range (so the rounding-error model holds)."""
    for a, zp in ((x, X_ZP), (y, Y_ZP)):
        if not np.array_equal(a, np.rint(a)):
            return False
        s = a - zp
        if np.abs(s).max() > 240.0:
            return False
    return True


def make_in_maps_fp8(x: np.ndarray, y: np.ndarray) -> list[dict]:
    """Shift, cast to fp8-e4m3 and pre-tile into the DoubleRow layouts.

    xt: per core [K, M_SH] -> [K/256, 128, 2, M_SH]
        (k = 256*g + 128*i + p  ->  [g, p, i, :])
    y:  per core [K, N_SH] -> [NB, K/256, 128, 2, NBW]  (same k bijection)
    """
    import ml_dtypes

    f8 = ml_dtypes.float8_e4m3
    xs = (np.ascontiguousarray(x, dtype=np.float32) - X_ZP).astype(f8)
    ys = (np.ascontiguousarray(y, dtype=np.float32) - Y_ZP).astype(f8)
    kq = K // 256
    nb_n = N_SH // NBW
    xt_shards = []
    for mi in range(MI):
        xt = xs[mi * M_SH : (mi + 1) * M_SH].T  # [K, M_SH]
        t = xt.reshape(kq, 2, 128, M_SH).transpose(0, 2, 1, 3)
        xt_shards.append(np.ascontiguousarray(t))
    y_shards = []
    for nj in range(NJ):
        yc = ys[:, nj * N_SH : (nj + 1) * N_SH]  # [K, N_SH]
        t = yc.reshape(kq, 2, 128, nb_n, NBW).transpose(3, 0, 2, 1, 4)
        y_shards.append(np.ascontiguousarray(t))
    return [{"xt": xt_shards[i // NJ], "y": y_shards[i % NJ]} for i in range(N_CORES)]


def make_in_maps(x: np.ndarray, y: np.ndarray) -> list[dict]:
    x = np.ascontiguousarray(x, dtype=np.float32)
    y = np.ascontiguousarray(y, dtype=np.float32)
    xt_shards = [
        np.ascontiguousarray(x[mi * M_SH : (mi + 1) * M_SH].T) for mi in range(MI)
    ]
    y_shards = [
        np.ascontiguousarray(y[:, nj * N_SH : (nj + 1) * N_SH]) for nj in range(NJ)
    ]
    return [{"xt": xt_shards[i // NJ], "y": y_shards[i % NJ]} for i in range(N_CORES)]


def make_in_maps_bf16(xb: np.ndarray, yb: np.ndarray) -> list[dict]:
    """Pre-tile bf16 shards to match build_bf16's DRAM layouts."""
    kp = K // 128
    nb_n = N_SH // NBW
    xt_shards = []
    for mi in range(MI):
        xt = xb[mi * M_SH : (mi + 1) * M_SH].T  # [K, M_SH]
        t = xt.reshape(kp // 2, 2, 128, M_SH).transpose(0, 2, 1, 3)
        xt_shards.append(np.ascontiguousarray(t))
    y_shards = []
    for nj in range(NJ):
        ys = yb[:, nj * N_SH : (nj + 1) * N_SH]  # [K, N_SH]
        t = ys.reshape(kp // 4, 4, 128, nb_n, NBW).transpose(3, 0, 2, 1, 4)
        y_shards.append(np.ascontiguousarray(t))
    return [{"xt": xt_shards[i // NJ], "y": y_shards[i % NJ]} for i in range(N_CORES)]


def _cast_bf16_exact(x: np.ndarray, y: np.ndarray):
    """Lossless repack to bf16 when every value survives the cast."""
    import ml_dtypes

    xb = np.ascontiguousarray(x, dtype=np.float32).astype(ml_dtypes.bfloat16)
    yb = np.ascontiguousarray(y, dtype=np.float32).astype(ml_dtypes.bfloat16)
    if np.array_equal(xb.astype(np.float32), x) and np.array_equal(
        yb.astype(np.float32), y
    ):
        return xb, yb
    return None


def _plan(x: np.ndarray, y: np.ndarray):
    """Pick the kernel variant + host-packed input maps for these inputs."""
    if _fp8_applicable(x, y):
        return "fp8", make_in_maps_fp8(x, y)
    casted = _cast_bf16_exact(x, y)
    if casted is not None:
        return "bf16", make_in_maps_bf16(*casted)
    return "f32", make_in_maps(x, y)


def kernel(x: np.ndarray, y: np.ndarray) -> np.ndarray:
    from concourse import bass_utils

    variant, in_maps = _plan(x, y)
    nc = _get_nc(variant)

    res = bass_utils.run_bass_kernel_spmd(nc, in_maps, core_ids=list(range(N_CORES)))

    out = np.empty((M, N), dtype=np.float32)
    for i in range(N_CORES):
        mi, nj = i // NJ, i % NJ
        o = res.results[i]["o"]
        if o.ndim == 4:  # [MO, NB, 128, NBW] pre-tiled layout
            o = o.transpose(0, 2, 1, 3).reshape(M_SH, N_SH)
        out[mi * M_SH : (mi + 1) * M_SH, nj * N_SH : (nj + 1) * N_SH] = o
    return out


# revision 45
# speedup vs baseline: 1.0089x; 1.0054x over previous
"""Dequantized mixed-sign int8 GEMM on 8 trn2 NeuronCores.

out = ((x - X_ZP) * X_SCALE) @ ((y - Y_ZP) * Y_SCALE)   [4096 x 4096 x 4096]

Strategy (fp8 path): the shifted operands (x+66 in [-62,193], y-160 in
[-160,95]) fit inside fp8-e4m3 range (+-240).  Rounding them to e4m3
costs ~2.3e-3 relative error on the output (vs the 2e-2 gate) because
the per-element relative rounding error (2^-4) averages down over the
K=4096 contraction while the output magnitude is dominated by the exact
mean term.  The GEMM then runs as fp8 x fp8 with DoubleRow perf mode
(two 128-row K-subtiles contracted per instruction, 157 TF/s = 2x bf16
peak) accumulating into fp32 PSUM; the scale product lands in the
PSUM->SBUF copy.  Measured ~127us vs the ~218us bf16 roofline
(~113us of pure matmul at fp8 peak + ~7us fixed NEFF preamble + ~3us
startup DMA latency + ~5us drain tail).

Sharding: 4-way over M x 2-way over N (core (mi, nj), mi in 0..3,
nj in 0..1).  Each core gets x[mi].T and y[:, nj] pre-shifted, cast to
fp8 and pre-tiled on the host into the DoubleRow SBUF layout
([128, 2, free] K-pair groups) so every DMA is one contiguous block and
there is zero on-chip elementwise work before the matmul.  x.T lives
SBUF-resident; y streams through in 512-wide blocks, double-buffered,
with 8 concurrent PSUM accumulation chains (one per PSUM bank).

Schedule notes (measured on hw, each worth ~1-2us):
- y loads ride the sync-engine HWDGE ring, x loads + output drains ride
  the ACT-engine ring, so descriptors issue in parallel and output-drain
  semaphore waits never stall the y stream.
- 5 dummy matmuls on a zeroed scratch tile warm the PE out of its low
  power-state during the otherwise-dead first-data window (the pstate
  decays when idle, so they must run back-to-back into the real stream).
- x group 0 splits in halves across both rings so the first matmul
  waits on 256KB, not 384KB.
- Block 0 interleaves all 8 chains per K step (the stream is input-
  paced); later blocks run chain-at-a-time so PSUM drains stagger.

Fallback: if the inputs are not integer-valued in the expected range
(so the fp8 error model would not hold), fall back to the exact bf16
kernel, and to a plain fp32 kernel if even bf16 would round.
"""

import sys

if "/opt/trn_rl_repo" not in sys.path:
    sys.path.insert(0, "/opt/trn_rl_repo")

import numpy as np

X_SCALE, X_ZP = 0.03, -66.0
Y_SCALE, Y_ZP = 0.025, 160.0
OUT_SCALE = float(np.float32(X_SCALE) * np.float32(Y_SCALE))

M = K = N = 4096
MI, NJ = 4, 2  # core grid: M split x N split
M_SH, N_SH = M // MI, N // NJ  # 1024, 2048 per core
N_CORES = MI * NJ
NBW = 512  # n-block width (one PSUM bank of fp32)


def build_fp8(m_sh=M_SH, n_sh=N_SH, k=K, nbw=NBW):
    """fp8-e4m3 DoubleRow GEMM.  Inputs arrive pre-shifted ((x - X_ZP),
    (y - Y_ZP)) and pre-tiled into K-pair groups [128, 2, free] so each
    matmul instruction contracts 256 K rows in 512 cycles (2x bf16
    FLOPs/cycle; measured 0.215us per [256K x 128M x 512N] matmul).
    No on-chip elementwise work: DVE only drains PSUM (scale by
    OUT_SCALE) and DMA moves raw blocks."""
    from concourse import bacc, mybir, tile

    f32, fp8 = mybir.dt.float32, mybir.dt.float8e4
    bf16 = mybir.dt.bfloat16
    DR = mybir.MatmulPerfMode.DoubleRow
    kq = k // 256  # K-pair groups (one DoubleRow matmul each)
    mo_n = m_sh // 128  # output chains (PSUM banks)
    nb_n = n_sh // nbw  # N blocks

    nc = bacc.Bacc("TRN2", target_bir_lowering=False, debug=False)
    xt_d = nc.dram_tensor("xt", (kq, 128, 2, m_sh), fp8, kind="ExternalInput")
    y_d = nc.dram_tensor("y", (nb_n, kq, 128, 2, nbw), fp8, kind="ExternalInput")
    # outputs ship bf16 (host upcasts): ~0.3us faster drain tail measured,
    # ~0.1% extra rounding against an 8.8x error-budget margin
    o_d = nc.dram_tensor("o", (mo_n, nb_n, 128, nbw), bf16, kind="ExternalOutput")

    with tile.TileContext(nc) as tc:
        with (
            tc.tile_pool(name="xp", bufs=1) as xp,
            tc.tile_pool(name="yp", bufs=3) as yp,
            tc.tile_pool(name="op", bufs=4) as op,
            tc.tile_pool(name="psum", bufs=1, space="PSUM") as pp,
        ):
            def load_y_tile(nb, g):
                yb = yp.tile([128, 2, nbw], fp8, tag=f"y{g}", name=f"y{nb}_{g}")
                nc.sync.dma_start(yb[:], y_d.ap()[nb, g])
                return yb

            # PE pstate warm-up: the tensor engine ramps from a low power
            # state and decays back when idle, so the dummies must run
            # back-to-back into the start of the real stream.  Fill the
            # dead window (first operands land ~1.8us after the first DMA
            # descriptor) with full-size dummy matmuls on a zeroed
            # scratch tile (into the ps7 bank, closed groups, numerically
            # inert) so the real stream starts already ramped.
            scratch = xp.tile([128, 2, nbw], fp8, tag="warm", name="warm")
            nc.gpsimd.memset(scratch[:], 0.0)
            ps_warm = pp.tile([128, nbw], f32, tag="ps7", name="ps7w")
            for _ in range(5):
                nc.tensor.matmul(
                    ps_warm[:],
                    scratch[:, :, :128],
                    scratch[:],
                    start=True,
                    stop=True,
                    perf_mode=DR,
                )

            # Startup stream: y descriptors issue on the sync ring while
            # x descriptors issue in parallel on the ACT ring (both are
            # HWDGE queues), so the first matmul's operands (y block-0
            # slice + resident x group 0) are in flight simultaneously
            # instead of serialized behind one descriptor queue.  The
            # g=0 x tile additionally splits across BOTH rings (half
            # right behind y block-0 g=0 on sync, half leading the ACT
            # ring) so its last byte lands ~1us sooner.  NOTE: tile
            # allocation order (y/x interleaved) is deliberate — grouping
            # all y tiles below all x tiles costs ~20% steady-state
            # matmul rate to SBUF bank conflicts.
            xbs = []
            y0 = []
            y0.append(load_y_tile(0, 0))
            x0a = xp.tile([128, 2, m_sh // 2], fp8, tag="x0a", name="x0a")
            nc.scalar.dma_start(x0a[:], xt_d.ap()[0][:, :, : m_sh // 2])
            x0b = xp.tile([128, 2, m_sh // 2], fp8, tag="x0b", name="x0b")
            nc.sync.dma_start(x0b[:], xt_d.ap()[0][:, :, m_sh // 2 :])
            xbs.append((x0a, x0b))
            for g in range(1, kq):
                y0.append(load_y_tile(0, g))
                xb = xp.tile([128, 2, m_sh], fp8, tag=f"x{g}", name=f"x{g}")
                nc.scalar.dma_start(xb[:], xt_d.ap()[g])
                xbs.append(xb)

            def x_slice(g, mo):
                if g == 0:
                    half = xbs[0][mo // 4]
                    return half[:, :, 128 * (mo % 4) : 128 * (mo % 4 + 1)]
                return xbs[g][:, :, 128 * mo : 128 * (mo + 1)]

            ybs_next = (
                [load_y_tile(1, g) for g in range(kq)] if nb_n > 1 else None
            )
            ybs = y0
            for nb in range(nb_n):
                pss = [
                    pp.tile([128, nbw], f32, tag=f"ps{mo}", name=f"ps{mo}")
                    for mo in range(mo_n)
                ]

                def copy_out(mo, nb=nb):
                    # output drains ride the ACT descriptor ring: their
                    # semaphore waits never stall the sync ring feeding
                    # the y stream, and the final drain's descriptor is
                    # pre-positioned on an otherwise idle queue
                    ot = op.tile([128, nbw], bf16, tag="ot", name="ot")
                    nc.vector.tensor_scalar_mul(ot[:], pss[mo][:], OUT_SCALE)
                    nc.scalar.dma_start(o_d.ap()[mo, nb], ot[:])

                def mm(mo, g, start=None, ybs=ybs):
                    nc.tensor.matmul(
                        pss[mo][:],
                        x_slice(g, mo),
                        ybs[g][:],
                        start=(g == 0) if start is None else start,
                        stop=(g == kq - 1),
                        perf_mode=DR,
                    )

                if nb == 0:
                    # block 0 is paced by the input stream: interleave all
                    # chains per K step so every arriving K-pair group
                    # unlocks mo_n matmuls for the in-order PE
                    for g in range(kq):
                        for mo in range(mo_n):
                            mm(mo, g)
                    for mo in range(mo_n):
                        copy_out(mo)
                else:
                    # data resident: run chains to completion one at a
                    # time so completions (and PSUM copies) stagger
                    # through the block instead of bursting at its end
                    for mo in range(mo_n):
                        for g in range(kq):
                            mm(mo, g)
                        copy_out(mo)
                ybs = ybs_next
                ybs_next = (
                    [load_y_tile(nb + 2, g) for g in range(kq)]
                    if nb + 2 < nb_n
                    else None
                )

    nc.compile()
    return nc


def build(m_sh=M_SH, n_sh=N_SH, k=K, nbw=NBW):
    """Fallback fp32 variant (used only if inputs are not even bf16-exact)."""
    from concourse import bacc, mybir, tile

    f32, bf16 = mybir.dt.float32, mybir.dt.bfloat16
    kp = k // 128  # K tiles of 128
    mo_n = m_sh // 128  # M tiles of 128
    nb_n = n_sh // nbw  # N blocks

    nc = bacc.Bacc("TRN2", target_bir_lowering=False, debug=False)
    xt_d = nc.dram_tensor("xt", (k, m_sh), f32, kind="ExternalInput")
    y_d = nc.dram_tensor("y", (k, n_sh), f32, kind="ExternalInput")
    o_d = nc.dram_tensor("o", (m_sh, n_sh), f32, kind="ExternalOutput")

    with tile.TileContext(nc) as tc:
        with (
            tc.tile_pool(name="xstage", bufs=3) as xstage,
            tc.tile_pool(name="ystage", bufs=8) as ystage,
            tc.tile_pool(name="xbf", bufs=1) as xbfp,
            tc.tile_pool(name="ybf", bufs=2) as ybfp,
            tc.tile_pool(name="opool", bufs=4) as opool,
            tc.tile_pool(name="psum", bufs=1, space="PSUM") as psum,
        ):
            def load_y(nb, ko):
                ys = ystage.tile([128, nbw], f32, tag="ys")
                nc.sync.dma_start(
                    ys[:],
                    y_d.ap()[128 * ko : 128 * (ko + 1), nb * nbw : (nb + 1) * nbw],
                )
                yb = ybfp.tile([128, nbw], bf16, tag=f"y{ko}")
                nc.vector.tensor_scalar_add(yb[:], ys[:], -Y_ZP)
                return yb

            xbf = []
            ybs0 = []
            for ko in range(kp):
                ybs0.append(load_y(0, ko))
                xs = xstage.tile([128, m_sh], f32, tag="xs")
                nc.sync.dma_start(xs[:], xt_d.ap()[128 * ko : 128 * (ko + 1), :])
                xb = xbfp.tile([128, m_sh], bf16, tag=f"x{ko}")
                nc.scalar.activation(
                    xb[:], xs[:], mybir.ActivationFunctionType.Copy, bias=-X_ZP
                )
                xbf.append(xb)

            for nb in range(nb_n):
                ybs = ybs0 if nb == 0 else [load_y(nb, ko) for ko in range(kp)]

                pss = [
                    psum.tile([128, nbw], f32, tag=f"ps{mo}", name=f"ps{mo}")
                    for mo in range(mo_n)
                ]
                for ko in range(kp):
                    for mo in range(mo_n):
                        nc.tensor.matmul(
                            pss[mo][:],
                            xbf[ko][:, 128 * mo : 128 * (mo + 1)],
                            ybs[ko][:],
                            start=(ko == 0),
                            stop=(ko == kp - 1),
                        )
                for mo in range(mo_n):
                    ot = opool.tile([128, nbw], f32, tag="ot")
                    nc.scalar.activation(
                        ot[:], pss[mo][:], mybir.ActivationFunctionType.Copy,
                        scale=OUT_SCALE,
                    )
                    nc.sync.dma_start(
                        o_d.ap()[128 * mo : 128 * (mo + 1), nb * nbw : (nb + 1) * nbw],
                        ot[:],
                    )

    nc.compile()
    return nc


def build_bf16(m_sh=M_SH, n_sh=N_SH, k=K, nbw=NBW):
    """bf16-input variant: exact for integer-valued quantized data; used
    as fallback when fp8's error model does not apply but bf16 casts are
    lossless."""
    from concourse import bacc, mybir, tile

    f32, bf16 = mybir.dt.float32, mybir.dt.bfloat16
    kp = k // 128
    mo_n = m_sh // 128
    nb_n = n_sh // nbw
    xg_n = kp // 2  # x groups: [128, 2, m_sh] (two K tiles per load)
    yg_n = kp // 4  # y groups: [128, 4, nbw] (four K tiles per load)

    nc = bacc.Bacc("TRN2", target_bir_lowering=False, debug=False)
    xt_d = nc.dram_tensor("xt", (xg_n, 128, 2, m_sh), bf16, kind="ExternalInput")
    y_d = nc.dram_tensor("y", (nb_n, yg_n, 128, 4, nbw), bf16, kind="ExternalInput")
    o_d = nc.dram_tensor("o", (mo_n, nb_n, 128, nbw), f32, kind="ExternalOutput")

    with tile.TileContext(nc) as tc:
        with (
            tc.tile_pool(name="xbf", bufs=1) as xbfp,
            tc.tile_pool(name="ybf", bufs=3) as ybfp,
            tc.tile_pool(name="opool", bufs=4) as opool,
            tc.tile_pool(name="psum", bufs=1, space="PSUM") as psum,
        ):
            x_at = [None] * kp
            y0_at = [None] * kp

            def load_x(g, eng_dve):
                xb = xbfp.tile([128, 2, m_sh], bf16, tag=f"x{g}", name=f"x{g}")
                nc.sync.dma_start(xb[:], xt_d.ap()[g])
                if eng_dve:
                    nc.vector.tensor_scalar_add(xb[:], xb[:], -X_ZP)
                else:
                    nc.scalar.activation(
                        xb[:], xb[:], mybir.ActivationFunctionType.Copy, bias=-X_ZP
                    )
                x_at[2 * g] = (xb, 0)
                x_at[2 * g + 1] = (xb, 1)

            def load_y(nb):
                tiles = []
                for g in range(yg_n):
                    yb = ybfp.tile(
                        [128, 4, nbw], bf16, tag=f"y{g}", name=f"y{nb}_{g}"
                    )
                    nc.sync.dma_start(yb[:], y_d.ap()[nb, g])
                    nc.vector.tensor_scalar_add(yb[:], yb[:], -Y_ZP)
                    tiles.append(yb)
                return tiles

            def x_slice(ko, mo):  # lhsT [128, 128]
                xb, j = x_at[ko]
                return xb[:, j, 128 * mo : 128 * (mo + 1)]

            def y_slice(ybs, ko):  # rhs [128, nbw]
                if ybs is None:
                    yb, j = y0_at[ko]
                    return yb[:, j, :]
                g, j = divmod(ko, 4)
                return ybs[g][:, j, :]

            def load_y0_part(j0, j1, tag):
                yb = ybfp.tile([128, j1 - j0, nbw], bf16, tag=tag, name=tag)
                nc.sync.dma_start(yb[:], y_d.ap()[0, 0][:, j0:j1, :])
                nc.vector.tensor_scalar_add(yb[:], yb[:], -Y_ZP)
                for j in range(j0, j1):
                    y0_at[j] = (yb, j - j0)

            def load_x0_part(j, tag, eng_dve):
                xb = xbfp.tile([128, 1, m_sh], bf16, tag=tag, name=tag)
                nc.sync.dma_start(xb[:], xt_d.ap()[0][:, j : j + 1, :])
                if eng_dve:
                    nc.vector.tensor_scalar_add(xb[:], xb[:], -X_ZP)
                else:
                    nc.scalar.activation(
                        xb[:], xb[:], mybir.ActivationFunctionType.Copy, bias=-X_ZP
                    )
                x_at[j] = (xb, 0)

            load_y0_part(0, 1, "y0a")
            load_x0_part(0, "x0a", eng_dve=True)
            load_x0_part(1, "x0b", eng_dve=False)
            if kp > 1:
                load_y0_part(1, min(4, kp), "y0b")
            if xg_n > 1:
                load_x(1, eng_dve=True)
            for gg in range(1, yg_n):
                yb = ybfp.tile([128, 4, nbw], bf16, tag=f"y{gg}", name=f"y0_{gg}")
                nc.sync.dma_start(yb[:], y_d.ap()[0, gg])
                nc.vector.tensor_scalar_add(yb[:], yb[:], -Y_ZP)
                for j in range(4):
                    y0_at[4 * gg + j] = (yb, j)
                for g in (2 * gg, 2 * gg + 1):
                    if g < xg_n:
                        load_x(g, eng_dve=(g % 2 == 0))

            ybs_next = load_y(1) if nb_n > 1 else None
            ybs = None
            for nb in range(nb_n):
                pss = [
                    psum.tile([128, nbw], f32, tag=f"ps{mo}", name=f"ps{mo}")
                    for mo in range(mo_n)
                ]

                def copy_out(mo, nb=nb):
                    ot = opool.tile([128, nbw], f32, tag="ot", name="ot")
                    use_dve = mo % 2 == 0 or (
                        nb == nb_n - 1 and mo == mo_n - 1
                    )
                    if use_dve:
                        nc.vector.tensor_scalar_mul(ot[:], pss[mo][:], OUT_SCALE)
                    else:
                        nc.scalar.activation(
                            ot[:], pss[mo][:], mybir.ActivationFunctionType.Copy,
                            scale=OUT_SCALE,
                        )
                    nc.sync.dma_start(o_d.ap()[mo, nb], ot[:])

                if nb == 0:
                    for ko in range(kp):
                        for mo in range(mo_n):
                            nc.tensor.matmul(
                                pss[mo][:],
                                x_slice(ko, mo),
                                y_slice(ybs, ko),
                                start=(ko == 0),
                                stop=(ko == kp - 1),
                            )
                    for mo in range(mo_n):
                        copy_out(mo)
                else:
                    for mo in range(mo_n):
                        for ko in range(kp):
                            nc.tensor.matmul(
                                pss[mo][:],
                                x_slice(ko, mo),
                                y_slice(ybs, ko),
                                start=(ko == 0),
                                stop=(ko == kp - 1),
                            )
                        copy_out(mo)
                ybs = ybs_next
                ybs_next = load_y(nb + 2) if nb + 2 < nb_n else None

    nc.compile()
    return nc


_nc_cache = {}


def _get_nc(variant="fp8"):
    if variant not in _nc_cache:
        builders = {"fp8": build_fp8, "bf16": build_bf16, "f32": build}
        _nc_cache[variant] = builders[variant]()
    return _nc_cache[variant]


def _fp8_applicable(x: np.ndarray, y: np.ndarray) -> bool:
    """fp8 path needs integer-valued inputs whose shifted values fit well
    inside e4m3 range (so the rounding-error model holds)."""
    for a, zp in ((x, X_ZP), (y, Y_ZP)):
        if not np.array_equal(a, np.rint(a)):
            return False
        s = a - zp
        if np.abs(s).max() > 240.0:
            return False
    return True


def make_in_maps_fp8(x: np.ndarray, y: np.ndarray) -> list[dict]:
    """Shift, cast to fp8-e4m3 and pre-tile into the DoubleRow layouts.

    xt: per core [K, M_SH] -> [K/256, 128, 2, M_SH]
        (k = 256*g + 128*i + p  ->  [g, p, i, :])
    y:  per core [K, N_SH] -> [NB, K/256, 128, 2, NBW]  (same k bijection)
    """
    import ml_dtypes

    f8 = ml_dtypes.float8_e4m3
    xs = (np.ascontiguousarray(x, dtype=np.float32) - X_ZP).astype(f8)
    ys = (np.ascontiguousarray(y, dtype=np.float32) - Y_ZP).astype(f8)
    kq = K // 256
    nb_n = N_SH // NBW
    xt_shards = []
    for mi in range(MI):
        xt = xs[mi * M_SH : (mi + 1) * M_SH].T  # [K, M_SH]
        t = xt.reshape(kq, 2, 128, M_SH).transpose(0, 2, 1, 3)
        xt_shards.append(np.ascontiguousarray(t))
    y_shards = []
    for nj in range(NJ):
        yc = ys[:, nj * N_SH : (nj + 1) * N_SH]  # [K, N_SH]
        t = yc.reshape(kq, 2, 128, nb_n, NBW).transpose(3, 0, 2, 1, 4)
        y_shards.append(np.ascontiguousarray(t))
    return [{"xt": xt_shards[i // NJ], "y": y_shards[i % NJ]} for i in range(N_CORES)]


def make_in_maps(x: np.ndarray, y: np.ndarray) -> list[dict]:
    x = np.ascontiguousarray(x, dtype=np.float32)
    y = np.ascontiguousarray(y, dtype=np.float32)
    xt_shards = [
        np.ascontiguousarray(x[mi * M_SH : (mi + 1) * M_SH].T) for mi in range(MI)
    ]
    y_shards = [
        np.ascontiguousarray(y[:, nj * N_SH : (nj + 1) * N_SH]) for nj in range(NJ)
    ]
    return [{"xt": xt_shards[i // NJ], "y": y_shards[i % NJ]} for i in range(N_CORES)]


def make_in_maps_bf16(xb: np.ndarray, yb: np.ndarray) -> list[dict]:
    """Pre-tile bf16 shards to match build_bf16's DRAM layouts."""
    kp = K // 128
    nb_n = N_SH // NBW
    xt_shards = []
    for mi in range(MI):
        xt = xb[mi * M_SH : (mi + 1) * M_SH].T  # [K, M_SH]
        t = xt.reshape(kp // 2, 2, 128, M_SH).transpose(0, 2, 1, 3)
        xt_shards.append(np.ascontiguousarray(t))
    y_shards = []
    for nj in range(NJ):
        ys = yb[:, nj * N_SH : (nj + 1) * N_SH]  # [K, N_SH]
        t = ys.reshape(kp // 4, 4, 128, nb_n, NBW).transpose(3, 0, 2, 1, 4)
        y_shards.append(np.ascontiguousarray(t))
    return [{"xt": xt_shards[i // NJ], "y": y_shards[i % NJ]} for i in range(N_CORES)]


def _cast_bf16_exact(x: np.ndarray, y: np.ndarray):
    """Lossless repack to bf16 when every value survives the cast."""
    import ml_dtypes

    xb = np.ascontiguousarray(x, dtype=np.float32).astype(ml_dtypes.bfloat16)
    yb = np.ascontiguousarray(y, dtype=np.float32).astype(ml_dtypes.bfloat16)
    if np.array_equal(xb.astype(np.float32), x) and np.array_equal(
        yb.astype(np.float32), y
    ):
        return xb, yb
    return None


def _plan(x: np.ndarray, y: np.ndarray):
    """Pick the kernel variant + host-packed input maps for these inputs."""
    if _fp8_applicable(x, y):
        return "fp8", make_in_maps_fp8(x, y)
    casted = _cast_bf16_exact(x, y)
    if casted is not None:
        return "bf16", make_in_maps_bf16(*casted)
    return "f32", make_in_maps(x, y)


def kernel(x: np.ndarray, y: np.ndarray) -> np.ndarray:
    from concourse import bass_utils

    variant, in_maps = _plan(x, y)
    nc = _get_nc(variant)

    res = bass_utils.run_bass_kernel_spmd(nc, in_maps, core_ids=list(range(N_CORES)))

    out = np.empty((M, N), dtype=np.float32)
    for i in range(N_CORES):
        mi, nj = i // NJ, i % NJ
        o = res.results[i]["o"]
        if o.ndim == 4:  # [MO, NB, 128, NBW] pre-tiled layout
            o = o.transpose(0, 2, 1, 3).reshape(M_SH, N_SH)
        out[mi * M_SH : (mi + 1) * M_SH, nj * N_SH : (nj + 1) * N_SH] = o
    return out
